# revision 45
# baseline (speedup 1.0000x reference)
"""DGN temporal GNN conv kernel for Trainium2 (8 NeuronCores).

Math (per timestep t):
    w_e(t) = edge_weight[e] if edge_time[e] <= node_time[t] else 0
    agg[n] = sum_{e: dst[e]==n} w_e(t) * x[t, src[e]]
    out[t] = agg @ W + b

node_time is sorted, so edge activity is monotone in t: edge e is active for
all t >= t_e where t_e = searchsorted(node_time, edge_time).

Device strategy:
  - Destination nodes sharded across 8 cores (6250 each); all timesteps on
    every core; one compile-time schedule (counts maxed over cores) -> one
    NEFF, SPMD.
  - Timesteps packed in QUADS: HBM tables xq[q] = [N, 256] bf16 rows holding
    x[4q..4q+3, n] (512B gather elements -> full DMA bus efficiency, one
    gather serves 4 timesteps).
  - Per (quad q, 64-dst group g, src half h): edges sorted by first-active
    timestep segment s = clip(t_e - 4q, 0, 3); per-segment capacities maxed
    over cores so the active edges at timestep 4q+j form a schedule-aligned
    PREFIX P[j] of the list.  Per 128-edge chunk ONE weighted one-hot sel
    (DVE tensor_scalar) serves all 4 timesteps; timestep j's matmul uses
    contraction rows 0:clip(P[j]-128c, 0, 128).
  - psum [64 feat, 4, 64 slot] accumulates the 4 timesteps of a group;
    drained once by ACT into aggT [64, 8, 6272] bf16 (feature-major).
  - Tail per timestep: aggT @ W (W stationary), +bias (ACT), stride-2
    PE-transposes so each staging partition holds a consecutive node pair
    (512B HBM runs -> no sub-512B DMA penalty).  Tail segments are emitted
    incrementally as their groups drain, overlapping later super-batches.
"""

import numpy as np

T, N, E, D = 8, 50000, 800000, 64
NC = 8
RANGE = N // NC          # 6250 dst nodes per core
GR = 64                  # nodes per group
NGRP = (RANGE + GR - 1) // GR   # 98 groups
SLOTS_PER_T = NGRP * GR  # 6272 aggT slots per timestep
NQ, QT = 2, 4            # 2 quads of 4 timesteps
SPLIT = 32768            # src split for int16 gather indices
CHUNK = 128              # edges per chunk (PE contraction dim)
SB_CHUNKS = 48           # max chunks per super-batch (msg SBUF tile size)
PAD_KEY = 99.0


# ---------------------------------------------------------------------------
# Host-side schedule
# ---------------------------------------------------------------------------

def _build_schedule(edge_index, edge_time, node_time, edge_weight):
    src = np.asarray(edge_index[0], dtype=np.int64)
    dst = np.asarray(edge_index[1], dtype=np.int64)
    edge_time = np.asarray(edge_time, dtype=np.float32)
    edge_weight = np.asarray(edge_weight, dtype=np.float32)
    node_time = np.asarray(node_time, dtype=np.float32)

    te = np.searchsorted(node_time, edge_time, side="left")  # first active t
    core = dst // RANGE
    g_all = (dst % RANGE) // GR
    slot_all = (dst % RANGE) % GR
    h_all = (src >= SPLIT).astype(np.int64)
    idx_all = np.where(h_all == 1, src - SPLIT, src)

    # caps[q, g, h, s]: max-over-cores count of edges in segment s
    caps = np.zeros((NQ, NGRP, 2, QT), dtype=np.int64)
    for q in range(NQ):
        m = te <= 4 * q + 3
        s = np.clip(te[m] - 4 * q, 0, None)
        bid = ((core[m] * NGRP + g_all[m]) * 2 + h_all[m]) * QT + s
        cnt = np.bincount(bid, minlength=NC * NGRP * 2 * QT)
        caps[q] = cnt.reshape(NC, NGRP, 2, QT).max(axis=0)
    caps[:, :, 0, 0] = np.maximum(caps[:, :, 0, 0], 1)  # lo prefix >= 1
    P = caps.cumsum(axis=3)                 # [NQ, NGRP, 2, QT] prefix slots
    L = P[:, :, :, QT - 1]                  # real slots per (q,g,h) list
    nch = -(-L // CHUNK)                    # chunks per (q,g,h)

    # Super-batches: consecutive groups of one quad, chunks <= SB_CHUNKS.
    # The final sb of each quad is kept small (2 groups) so the last
    # drain->tail chain at the end of the quad is short.
    parts = []
    for q in range(NQ):
        cur, tot = [], 0
        for g in range(NGRP):
            c = int(nch[q, g, 0] + nch[q, g, 1])
            if cur and tot + c > SB_CHUNKS:
                parts.append((q, cur))
                cur, tot = [], 0
            cur.append(g)
            tot += c
        if len(cur) > 2:
            parts.append((q, cur[:-2]))
            parts.append((q, cur[-2:]))
        else:
            parts.append((q, cur))
    sbs = []
    chunk_base = np.zeros((NQ, 2, NGRP), dtype=np.int64)
    n_chunks = 0
    for q, groups in parts:
        lo0 = n_chunks
        for gg in groups:
            chunk_base[q, 0, gg] = n_chunks
            n_chunks += int(nch[q, gg, 0])
        hi0 = n_chunks
        for gg in groups:
            chunk_base[q, 1, gg] = n_chunks
            n_chunks += int(nch[q, gg, 1])
        sbs.append({"q": q, "groups": groups,
                    "lo": (lo0, hi0), "hi": (hi0, n_chunks)})
    n_slots = n_chunks * CHUNK

    # seg_off[q,g,h,s]: slot offset of segment s within its (q,g,h) list
    seg_off = P - caps

    idx_stream = np.zeros((NC, n_slots), dtype=np.int16)
    key_stream = np.full((NC, n_slots), PAD_KEY, dtype=np.float32)
    w_stream = np.zeros((NC, n_slots), dtype=np.float32)
    for q in range(NQ):
        m = te <= 4 * q + 3
        s = np.clip(te[m] - 4 * q, 0, None)
        k_ = core[m]
        gg = g_all[m]
        hh = h_all[m]
        ii = idx_all[m]
        ss = slot_all[m]
        ww = edge_weight[m]
        order = np.lexsort((s, hh, gg, k_))
        k_, gg, hh, s, ii, ss, ww = (a[order] for a in
                                     (k_, gg, hh, s, ii, ss, ww))
        # rank within each (k,g,h,s) bin
        bid = ((k_ * NGRP + gg) * 2 + hh) * QT + s
        cnts = np.bincount(bid, minlength=NC * NGRP * 2 * QT)
        starts = np.concatenate([[0], np.cumsum(cnts)[:-1]])
        r = np.arange(len(bid)) - starts[bid]
        slotpos = (chunk_base[q, hh, gg] * CHUNK
                   + seg_off[q, gg, hh, s] + r)
        idx_stream[k_, slotpos] = ii.astype(np.int16)
        key_stream[k_, slotpos] = ss.astype(np.float32)
        w_stream[k_, slotpos] = ww

    sched = {"sbs": sbs, "nch": nch, "chunk_base": chunk_base, "P": P,
             "n_chunks": n_chunks, "n_slots": n_slots}
    return sched, (idx_stream, key_stream, w_stream)


def _pack_idx(idx_stream):
    """[NC, n_slots] -> [NC, 128, n_slots//16]: slot j at partition j%16,
    col j//16, replicated into all 8 groups of 16 partitions."""
    nc_, n_slots = idx_stream.shape
    cols = n_slots // 16
    wrapped = idx_stream.reshape(nc_, cols, 16).transpose(0, 2, 1)
    return np.ascontiguousarray(np.tile(wrapped, (1, 8, 1)))


def _chunk_plan(sched, q, g):
    """Per (q,g): ordered chunk list [(h, c_in_list, ci_global)], and for
    each chunk the rows k_j per timestep j plus start/stop flags."""
    nch = sched["nch"]
    chunk_base = sched["chunk_base"]
    P = sched["P"]
    seq = []
    for h in (0, 1):
        for c in range(int(nch[q, g, h])):
            seq.append((h, c, int(chunk_base[q, h, g]) + c))
    plan = []
    last_for_j = {}
    for pos, (h, c, ci) in enumerate(seq):
        ks = []
        for j in range(QT):
            k = int(min(max(P[q, g, h, j] - CHUNK * c, 0), CHUNK))
            ks.append(k)
            if k > 0:
                last_for_j[j] = pos
        plan.append([h, c, ci, ks])
    return plan, last_for_j


# ---------------------------------------------------------------------------
# Numpy emulation of the device schedule (host-logic validation)
# ---------------------------------------------------------------------------

def emulate(x, edge_index, edge_time, node_time, edge_weight, W, b):
    sched, (idx_s, key_s, w_s) = _build_schedule(
        edge_index, edge_time, node_time, edge_weight)
    xf = np.asarray(x, dtype=np.float32)
    Wf = np.asarray(W, dtype=np.float32)
    bf = np.asarray(b, dtype=np.float32)
    out = np.zeros((T, N, D), dtype=np.float32)
    iota = np.arange(GR, dtype=np.float32)
    for k in range(NC):
        aggT = np.zeros((D, T, SLOTS_PER_T), dtype=np.float32)
        for q in range(NQ):
            for g in range(NGRP):
                plan, _ = _chunk_plan(sched, q, g)
                psum = np.zeros((QT, D, GR), dtype=np.float32)
                for h, c, ci, ks in plan:
                    sl = slice(ci * CHUNK, (ci + 1) * CHUNK)
                    idx = idx_s[k, sl].astype(np.int64)
                    key = key_s[k, sl]
                    w = w_s[k, sl]
                    sel = (key[:, None] == iota[None, :]) * w[:, None]
                    base = SPLIT if h else 0
                    for j in range(QT):
                        kr = ks[j]
                        if kr == 0:
                            continue
                        msg = xf[4 * q + j, base + idx[:kr], :]
                        psum[j] += msg.T @ sel[:kr]
                for j in range(QT):
                    out_sl = slice(g * GR, (g + 1) * GR)
                    aggT[:, 4 * q + j, out_sl] = psum[j]
        for t in range(T):
            outT = Wf.T @ aggT[:, t, :] + bf[:, None]
            out[t, k * RANGE:(k + 1) * RANGE, :] = outT[:, :RANGE].T
    return out


# ---------------------------------------------------------------------------
# Bass kernel builder
# ---------------------------------------------------------------------------

def build_tile_kernel(tc, out_ap, ins, sched):
    """ins: xq0/xq1 [N, 256] bf16, idx [128, n_slots//16] i16,
    key/wgt [128, n_chunks] f32, iota [128, 64] bf16, wmat [64, 64] bf16,
    bias [64, 1] f32, ident [64, 64] f32.  out_ap: [T*RANGE, 64] f32."""
    from contextlib import ExitStack
    from concourse import bass, tile, mybir
    dt = mybir.dt
    nc = tc.nc
    nch = sched["nch"]
    chunk_base = sched["chunk_base"]

    with ExitStack() as ctx:
        const_p = ctx.enter_context(tc.tile_pool(name="const", bufs=1))
        msg_p = ctx.enter_context(tc.tile_pool(name="msg", bufs=3))
        aux_p = ctx.enter_context(tc.tile_pool(name="aux", bufs=8))
        sel_p = ctx.enter_context(tc.tile_pool(name="sel", bufs=8))
        agg_p = ctx.enter_context(tc.tile_pool(name="agg", bufs=1))
        stage_p = ctx.enter_context(tc.tile_pool(name="stage", bufs=4))
        psum_p = ctx.enter_context(tc.tile_pool(name="psum", bufs=4, space="PSUM"))
        psumt_p = ctx.enter_context(tc.tile_pool(name="psumt", bufs=2, space="PSUM"))

        iota_t = const_p.tile([128, GR], dt.bfloat16, tag="iota")
        nc.sync.dma_start(iota_t[:], ins["iota"][:])
        wmat_t = const_p.tile([128, D], dt.bfloat16, tag="wmat")
        nc.sync.dma_start(wmat_t[:], ins["wmat"][:])
        bias_t = const_p.tile([D, 1], dt.float32, tag="bias")
        nc.sync.dma_start(bias_t[:], ins["bias"][:])
        ident_t = const_p.tile([D, D], dt.float32, tag="ident")
        nc.sync.dma_start(ident_t[:], ins["ident"][:])

        aggT = agg_p.tile([D, T, SLOTS_PER_T], dt.bfloat16, tag="aggT")

        xq = [ins["xq0"], ins["xq1"]]

        def emit_tail_seg(t, s0):
            """One 512-slot tail segment: @W, +bias, pair-interleaved
            transposes (partition p holds nodes 2p/2p+1 -> 512B HBM runs)."""
            q, j = t // QT, t % QT
            w512 = min(512, SLOTS_PER_T - s0)
            psw = psumt_p.tile([D, 512], dt.float32, tag="psw")
            nc.tensor.matmul(psw[:, :w512], wmat_t[0:64, :],
                             aggT[:, 4 * q + j, s0:s0 + w512],
                             start=True, stop=True)
            outTs = stage_p.tile([D, 512], dt.float32, tag="outTs")
            nc.scalar.activation(outTs[:, :w512], psw[:, :w512],
                                 mybir.ActivationFunctionType.Identity,
                                 bias=bias_t[:])
            pst = psumt_p.tile([128, 8, D], dt.float32, tag="pst")
            # stride-2 transposes: partition p holds node pair (2p, 2p+1)
            # so each HBM run is 512B (no sub-512B DMA penalty)
            nb2 = w512 // 256          # full 256-col pair-blocks
            st = stage_p.tile([128, 4, D], dt.float32, tag="st")
            if nb2 > 0:
                ovv = outTs[:, :nb2 * 256].rearrange(
                    "p (b x r) -> p b x r", b=nb2, r=2)
                for b in range(nb2):
                    for r in range(2):
                        nc.tensor.transpose(pst[:, 2 * b + r, :],
                                            ovv[:, b, :, r], ident_t[:])
                nc.vector.tensor_copy(st[:, :2 * nb2, :], pst[:, :2 * nb2, :])
                # dst rows s0 + 256b + 2p + r  <-  st[p, (b r), f]
                dst = out_ap[t * RANGE + s0:t * RANGE + s0 + 512, :]
                dst = dst.rearrange("(b p r) f -> p b r f", b=nb2, r=2)
                nc.sync.dma_start(dst, st[:, :2 * nb2, :])
            else:
                # final partial segment: w512 == 128 cols = 64 node-pairs;
                # 106 valid rows = 53 complete pairs
                ovv = outTs[:, :w512].rearrange("p (x r) -> p x r", r=2)
                for r in range(2):
                    nc.tensor.transpose(pst[0:64, r, :], ovv[:, :, r],
                                        ident_t[:])
                nc.vector.tensor_copy(st[0:64, :2, :], pst[0:64, :2, :])
                nrow = RANGE - s0
                dst = out_ap[t * RANGE + s0:t * RANGE + s0 + nrow, :]
                dst = dst.rearrange("(p r) f -> p r f", r=2)
                nc.sync.dma_start(dst, st[0:nrow // 2, :2, :])

        n_sbs = len(sched["sbs"])
        tail_next = [0] * NQ     # next un-emitted tail slot offset per quad
        for sb_i, sb in enumerate(sched["sbs"]):
            q = sb["q"]
            lo0, lo1 = sb["lo"]
            hi0, hi1 = sb["hi"]
            nb = hi1 - lo0
            msg = msg_p.tile([128, SB_CHUNKS, 2 * CHUNK], dt.bfloat16,
                             tag="msg")
            for (c0, c1, base) in ((lo0, lo1, 0), (hi0, hi1, SPLIT)):
                nchk = c1 - c0
                if nchk == 0:
                    continue
                nidx = nchk * CHUNK
                idx_t = aux_p.tile([128, SB_CHUNKS * 8], dt.int16, tag="idx")
                nc.sync.dma_start(idx_t[:, :nidx // 16],
                                  ins["idx"][:, c0 * 8:c0 * 8 + nidx // 16])
                src_ap = xq[q][SPLIT:N, :] if base else xq[q][0:SPLIT, :]
                nc.gpsimd.dma_gather(
                    out_ap=msg[:, c0 - lo0:c0 - lo0 + nchk, :],
                    in_ap=src_ap,
                    idxs_ap=idx_t[:, :nidx // 16],
                    num_idxs=nidx,
                    num_idxs_reg=nidx,
                    elem_size=2 * CHUNK,
                    single_packet=False,
                )
            key_t = aux_p.tile([128, SB_CHUNKS], dt.float32, tag="key")
            nc.sync.dma_start(key_t[:, :nb], ins["key"][:, lo0:lo0 + nb])
            w_t = aux_p.tile([128, SB_CHUNKS], dt.float32, tag="wgt")
            nc.sync.dma_start(w_t[:, :nb], ins["wgt"][:, lo0:lo0 + nb])

            for g in sb["groups"]:
                plan, last_for_j = _chunk_plan(sched, q, g)
                # psum partitions 0:64 = even-j feats, 64:128 = odd-j feats;
                # col block p = timestep pair (2p, 2p+1).  Full 2KB PSUM
                # bank: start=True pending-zeroes the whole bank, so it is
                # issued exactly once (first matmul); each region's first
                # start=False write then overwrites its still-pending bytes,
                # later ones accumulate.
                psum = psum_p.tile([D, 2 * QT, GR], dt.float32, tag="grp")
                n_pos = len(plan)
                for pos, (h, c, ci, ks) in enumerate(plan):
                    sel = sel_p.tile([128, GR], dt.bfloat16, tag="sel")
                    nc.vector.tensor_scalar(
                        sel[:], iota_t[:],
                        key_t[:, ci - lo0:ci - lo0 + 1],
                        w_t[:, ci - lo0:ci - lo0 + 1],
                        mybir.AluOpType.is_equal, mybir.AluOpType.mult)
                    for j in range(QT):
                        kr = ks[j]
                        if kr == 0:
                            continue
                        nc.tensor.matmul(
                            psum[:, j, :],
                            msg[0:kr, ci - lo0, j * D:(j + 1) * D],
                            sel[0:kr, :],
                            start=(pos == 0 and j == 0),
                            stop=(pos == n_pos - 1 and j == QT - 1))
                nc.scalar.activation(aggT[:, 4 * q:4 * q + QT,
                                          g * GR:(g + 1) * GR],
                                     psum[:, 0:QT, :],
                                     mybir.ActivationFunctionType.Copy)

            # incremental tail: emit 512-slot segments whose groups have all
            # been drained, so tail work overlaps later super-batches
            last_of_quad = (sb_i == n_sbs - 1
                            or sched["sbs"][sb_i + 1]["q"] != q)
            drained = SLOTS_PER_T if last_of_quad else (sb["groups"][-1] + 1) * GR
            while tail_next[q] + 512 <= drained or (
                    drained == SLOTS_PER_T and tail_next[q] < SLOTS_PER_T):
                s0 = tail_next[q]
                for t in range(4 * q, 4 * q + QT):
                    emit_tail_seg(t, s0)
                tail_next[q] += 512


# ---------------------------------------------------------------------------
# Top-level kernel
# ---------------------------------------------------------------------------

_CACHE = {}


def _declare_io(nc, dt, n_chunks, n_slots, null=False):
    in_aps = {}
    for q in range(NQ):
        in_aps[f"xq{q}"] = nc.dram_tensor(
            f"xq{q}", [N, 2 * CHUNK], dt.bfloat16, kind="ExternalInput").ap()
    in_aps["idx"] = nc.dram_tensor(
        "idx", [128, n_slots // 16], dt.int16, kind="ExternalInput").ap()
    in_aps["key"] = nc.dram_tensor(
        "key", [128, n_chunks], dt.float32, kind="ExternalInput").ap()
    in_aps["wgt"] = nc.dram_tensor(
        "wgt", [128, n_chunks], dt.float32, kind="ExternalInput").ap()
    in_aps["iota"] = nc.dram_tensor(
        "iota", [128, GR], dt.bfloat16, kind="ExternalInput").ap()
    in_aps["wmat"] = nc.dram_tensor(
        "wmat", [128, D], dt.bfloat16, kind="ExternalInput").ap()
    in_aps["bias"] = nc.dram_tensor(
        "bias", [D, 1], dt.float32, kind="ExternalInput").ap()
    in_aps["ident"] = nc.dram_tensor(
        "ident", [D, D], dt.float32, kind="ExternalInput").ap()
    shape = [128, D] if null else [T * RANGE, D]
    out_ap = nc.dram_tensor("out", shape, dt.float32, kind="ExternalOutput").ap()
    return in_aps, out_ap


def _get_state(edge_index, edge_time, node_time, edge_weight):
    from concourse import bacc, tile, mybir
    dt = mybir.dt
    key = (edge_index.tobytes(), edge_time.tobytes(), node_time.tobytes(),
           edge_weight.tobytes())
    key = hash(key)
    if _CACHE.get("key") == key:
        return _CACHE["state"]

    sched, (idx_s, key_s, w_s) = _build_schedule(
        edge_index, edge_time, node_time, edge_weight)
    n_chunks, n_slots = sched["n_chunks"], sched["n_slots"]

    nc = bacc.Bacc("TRN2", target_bir_lowering=False, debug=False,
                   enable_asserts=False)
    in_aps, out_ap = _declare_io(nc, dt, n_chunks, n_slots)
    with tile.TileContext(nc) as tc:
        build_tile_kernel(tc, out_ap, in_aps, sched)
    if not nc.is_finalized():
        nc.finalize()

    # Null kernel: same inputs, trivial body (for transfer-overhead baseline).
    nc0 = bacc.Bacc("TRN2", target_bir_lowering=False, debug=False,
                    enable_asserts=False)
    in_aps0, out_ap0 = _declare_io(nc0, dt, n_chunks, n_slots, null=True)
    with tile.TileContext(nc0) as tc0:
        from contextlib import ExitStack
        with ExitStack() as c0:
            p0 = c0.enter_context(tc0.tile_pool(name="p0", bufs=1))
            t0_ = p0.tile([128, D], dt.float32, tag="t0")
            nc0.vector.memset(t0_[:], 0.0)
            nc0.sync.dma_start(t0_[0:D, :], in_aps0["ident"][:])
            nc0.sync.dma_start(out_ap0[:], t0_[:])
    if not nc0.is_finalized():
        nc0.finalize()

    state = {"sched": sched, "idx_s": idx_s, "key_s": key_s, "w_s": w_s,
             "nc": nc, "nc0": nc0,
             "idx_packed": _pack_idx(idx_s),
             "key_packed": key_s.reshape(NC, n_chunks, CHUNK)
                                .transpose(0, 2, 1).copy(),
             "w_packed": w_s.reshape(NC, n_chunks, CHUNK)
                            .transpose(0, 2, 1).copy()}
    _CACHE["key"] = key
    _CACHE["state"] = state
    return state


def _make_in_maps(state, x, W, b):
    import ml_dtypes
    bf16 = ml_dtypes.bfloat16
    xb = np.asarray(x).astype(bf16)                       # [T, N, 64]
    xqs = [np.ascontiguousarray(
               xb[4 * q:4 * q + QT].transpose(1, 0, 2).reshape(N, QT * D))
           for q in range(NQ)]                            # [N, 256] each
    iota_np = np.tile(np.arange(GR, dtype=np.float32)[None, :],
                      (128, 1)).astype(bf16)
    wmat_np = np.tile(np.asarray(W).astype(bf16), (2, 1))
    bias_np = np.asarray(b).astype(np.float32).reshape(D, 1)
    ident_np = np.eye(D, dtype=np.float32)
    in_maps = []
    for k in range(NC):
        m = {f"xq{q}": xqs[q] for q in range(NQ)}
        m["idx"] = state["idx_packed"][k]
        m["key"] = state["key_packed"][k]
        m["wgt"] = state["w_packed"][k]
        m["iota"] = iota_np
        m["wmat"] = wmat_np
        m["bias"] = bias_np
        m["ident"] = ident_np
        in_maps.append(m)
    return in_maps


def kernel(x, edge_index, edge_time, node_time, edge_weight, W, b):
    from concourse.bass_utils import run_bass_kernel_spmd
    edge_index = np.asarray(edge_index)
    edge_time = np.asarray(edge_time)
    node_time = np.asarray(node_time)
    edge_weight = np.asarray(edge_weight)
    state = _get_state(edge_index, edge_time, node_time, edge_weight)
    in_maps = _make_in_maps(state, x, W, b)
    res = run_bass_kernel_spmd(state["nc"], in_maps, core_ids=list(range(NC)))
    out = np.zeros((T, N, D), dtype=np.float32)
    for k in range(NC):
        o = res.results[k]["out"].reshape(T, RANGE, D)
        out[:, k * RANGE:(k + 1) * RANGE, :] = o
    _CACHE["last_results"] = res
    return out


def null_run(x, edge_index, edge_time, node_time, edge_weight, W, b):
    """Same input transfer volume, trivial compute (timing baseline)."""
    from concourse.bass_utils import run_bass_kernel_spmd
    state = _get_state(np.asarray(edge_index), np.asarray(edge_time),
                       np.asarray(node_time), np.asarray(edge_weight))
    in_maps = _make_in_maps(state, x, W, b)
    res = run_bass_kernel_spmd(state["nc0"], in_maps, core_ids=list(range(NC)))
    return res.results[0]["out"]
